# revision 31
# baseline (speedup 1.0000x reference)
"""3-layer GAT (GATConv x3 + FC) on 8 Trainium2 NeuronCores.

Strategy: dst-sorted edge partitioning (each core owns a contiguous node range
and all edges into it), per-layer node-parallel feature matmul + AllGather of a
gatherable node table [h | e_src], then an edge phase per core: batched
dma_gather of src rows (int16 indices, table split in two halves to fit int16),
exp(leaky(e_src+e_dst)) edge weights, and segment reduction via one-hot
selection-matrix matmuls accumulated in PSUM per 128-node chunk. Softmax is
unnormalized (exp without max subtraction) with per-node post-normalization by
the gathered weight sum; self-loops guarantee the sum is bounded away from 0.

Gathers are descriptor-rate-bound on TRN2 (~4 ns/row across 4 SWDGE queues,
payload size irrelevant), so the design batches thousands of rows per
dma_gather instruction and spreads calls across all 4 queues.
"""
import os, sys
sys.path.insert(0, '/opt/trn_rl_repo')
import math
import numpy as np
ATH_STAGE = int(os.environ.get("ATH_STAGE", "6"))
ATH_EDGE = int(os.environ.get("ATH_EDGE", "4"))
CPB = int(os.environ.get("ATH_CPB", "2"))
import ml_dtypes

import concourse.bass as bass
import concourse.bacc as bacc
import concourse.mybir as mybir
import concourse.tile as tile
from concourse import bass_utils
from concourse.bass import _add_dep_helper

# ---- model constants (must match reference.py) ----
NEG_SLOPE = 0.2
H1, H2, H3 = 4, 4, 1
CH = 64
N_NODES = 50000
N_EDGES = 800000
IN_DIM = 128
N_CLASSES = 10

W = 8                    # cores
OWN = 6272               # nodes per core (49 chunks of 128)
NPAD = W * OWN           # 50176
NCHUNK = OWN // 128      # 49
NWIN = 2                 # 64-node windows per chunk
WINW = 64
PAD_SEG = 99.0
NQ = 4                   # SWDGE queues
# table is AllGathered in two halves (per-core local rows [0,LSPL) / [LSPL,OWN))
# so each half-table's row index fits int16 and AG_a overlaps the node phase
LSPL = 3200              # local split row (25 chunks)
RA = W * LSPL            # 25600 rows in tableA
RB = W * (OWN - LSPL)    # 24576 rows in tableB

# table row layout (bf16 slots)
ROW12 = 384              # layers 1/2: [h(256) | e_src f32 (8 slots) | pad] = 768B stride
PAY12 = 264
ROW3 = 128               # layer 3: [h(64) | e_src f32 (2 slots) | pad] = 256B stride
PAY3 = 66
EROW = 128               # e_dst local table row: 256B stride, leading slots f32

dt = mybir.dt
AF = mybir.ActivationFunctionType
OP = mybir.AluOpType
bf16 = ml_dtypes.bfloat16

# relax dma_gather's elem_size%256 assert (q7 ucode only requires the row
# stride to be a multiple of 256B; elem_size is the per-descriptor length)
import inspect as _inspect, textwrap as _textwrap
if not getattr(bass, "_ath_dma_gather_patched", False):
    _src = _textwrap.dedent(_inspect.getsource(bass.BassGpSimd.dma_gather))
    _src = _src.replace("elem_size_bytes > 0 and elem_size_bytes % 256 == 0",
                        "elem_size_bytes > 0")
    _ns = dict(bass.__dict__)
    exec(compile(_src, "<dma_gather_patched>", "exec"), _ns)
    bass.BassGpSimd.dma_gather = _ns["dma_gather"]
    bass._ath_dma_gather_patched = True


def _wrap_idxs(idx_i16):
    """dma_gather index layout: idx i at [i%16, i//16], replicated to 128 parts."""
    n = idx_i16.shape[0]
    arr = idx_i16.reshape(n // 16, 16).T.copy()
    return np.tile(arr, (8, 1))


def _host_prep(x, edge_index):
    """Edge sorting/tiling; returns per-core input arrays + static tile metadata.

    Stream layout per block (CPB chunks): all lo-half tiles (grouped per
    (chunk, window)), then all hi-half tiles. Each (c, w, half) subgroup is
    padded to a multiple of 128 edges; pad slots gather row 0 of their half
    and carry PAD_SEG so the one-hot matrix zeroes their contribution.
    """
    src = np.concatenate([edge_index[0].astype(np.int64),
                          np.arange(N_NODES, dtype=np.int64)])
    dst = np.concatenate([edge_index[1].astype(np.int64),
                          np.arange(N_NODES, dtype=np.int64)])
    order = np.argsort(dst, kind="stable")
    src, dst = src[order], dst[order]

    core = dst // OWN
    chunk = (dst % OWN) // 128
    win = ((dst % OWN) % 128) // WINW
    half = ((src % OWN) >= LSPL).astype(np.int64)
    # half-table row index: A: core*LSPL + local ; B: core*(OWN-LSPL) + local-LSPL
    htrow = np.where(half == 0,
                     (src // OWN) * LSPL + (src % OWN),
                     (src // OWN) * (OWN - LSPL) + (src % OWN) - LSPL)

    # group key: (core, chunk, win, half)
    key = ((core * NCHUNK + chunk) * NWIN + win) * 2 + half
    korder = np.argsort(key, kind="stable")
    src_s, dst_s, htrow_s = src[korder], dst[korder], htrow[korder]
    ngroups = W * NCHUNK * NWIN * 2
    counts = np.bincount(key[korder], minlength=ngroups).reshape(W, NCHUNK, NWIN, 2)
    starts = np.zeros(ngroups + 1, dtype=np.int64)
    np.cumsum(counts.reshape(-1), out=starts[1:])

    # tiles per (chunk, win, half): shared across cores (SPMD program)
    tiles_per = np.ceil(counts / 128.0).astype(np.int64).max(axis=0)  # [NCHUNK, NWIN, 2]
    tiles_per = np.maximum(tiles_per, 1)

    blocks = []
    c0 = 0
    while c0 < NCHUNK:
        nb = min(CPB, NCHUNK - c0)
        tl = []            # tile list in stream order: (cl, w, half)
        for h_ in range(2):
            for cl in range(nb):
                for w_ in range(NWIN):
                    for _ in range(int(tiles_per[c0 + cl, w_, h_])):
                        tl.append((cl, w_, h_))
        nlo = sum(1 for t in tl if t[2] == 0)
        nhi = len(tl) - nlo
        blocks.append(dict(c0=c0, nb=nb, tl=tl, nlo=nlo, nhi=nhi, Tall=len(tl)))
        c0 += nb

    per_core = []
    for r in range(W):
        glo_cols, ghi_cols, eid_cols, seg_cols = [], [], [], []
        for b in blocks:
            # consume each (cl, w, half) group's edges across its tiles
            used = {}
            for (cl, w_, h_) in b["tl"]:
                c = b["c0"] + cl
                g = ((r * NCHUNK + c) * NWIN + w_) * 2 + h_
                k0 = used.get((cl, w_, h_), 0)
                used[(cl, w_, h_)] = k0 + 1
                s0 = starts[g] + k0 * 128
                n_real = max(0, min(128, (starts[g + 1] - starts[g]) - k0 * 128))
                sl_src = htrow_s[s0:s0 + n_real]
                sl_dst = dst_s[s0:s0 + n_real]
                pad = 128 - n_real
                gi = np.concatenate([sl_src, np.zeros(pad, np.int64)])
                ei = np.concatenate([sl_dst - r * OWN, np.zeros(pad, np.int64)])
                sg = np.concatenate(
                    [(sl_dst - r * OWN - c * 128 - w_ * WINW).astype(np.float64),
                     np.full(pad, PAD_SEG)])
                (glo_cols if h_ == 0 else ghi_cols).append(gi)
                eid_cols.append(ei)
                seg_cols.append(sg)
        # gidx: per block, wrapped lo segment then wrapped hi segment
        g_parts = []
        gpos = hpos = 0
        for b in blocks:
            n_lo_t = sum(1 for t in b["tl"] if t[2] == 0)
            n_hi_t = b["Tall"] - n_lo_t
            lo = np.concatenate(glo_cols[gpos:gpos + n_lo_t]) if n_lo_t else np.zeros(0, np.int64)
            hi = np.concatenate(ghi_cols[hpos:hpos + n_hi_t]) if n_hi_t else np.zeros(0, np.int64)
            gpos += n_lo_t
            hpos += n_hi_t
            if n_lo_t:
                g_parts.append(_wrap_idxs(lo.astype(np.int16)))
            if n_hi_t:
                g_parts.append(_wrap_idxs(hi.astype(np.int16)))
        gidx = np.concatenate(g_parts, axis=1)
        # seg: per block contiguous in (partition, tile) layout
        s_parts = []
        pos = 0
        for b in blocks:
            T = b["Tall"]
            s_flat = np.concatenate(seg_cols[pos:pos + T])
            pos += T
            s_parts.append(np.ascontiguousarray(
                s_flat.reshape(-1, 128).T.astype(bf16)))
        seg = np.concatenate(s_parts, axis=1)
        # segT: seg in flat stream order, replicated across 64 partitions
        # (feeds the S^T one-hot build for the e_dst delivery matmuls)
        segf = np.concatenate(seg_cols)
        segT = np.broadcast_to(segf.astype(bf16)[None, :], (64, segf.shape[0]))
        per_core.append(dict(gidx=np.ascontiguousarray(gidx),
                             seg=np.ascontiguousarray(seg),
                             segT=np.ascontiguousarray(segT)))
    return blocks, per_core


def _build_program(blocks, heads_cfg):
    """Build the full 8-core SPMD Bass program."""
    nc = bacc.Bacc("TRN2", target_bir_lowering=False, debug=False, num_devices=W,
                   num_swdge_queues=NQ)
    rg = [list(range(W))]

    # ---------------- inputs ----------------
    pc0_shapes = heads_cfg["pc_shapes"]
    x1T_d = nc.dram_tensor("x1T", [128, OWN], dt.float32, kind="ExternalInput")
    gidx_d = nc.dram_tensor("gidx", list(pc0_shapes["gidx"]), dt.int16, kind="ExternalInput")
    segT_d = nc.dram_tensor("segT", list(pc0_shapes["segT"]), dt.bfloat16, kind="ExternalInput")
    seg_d = nc.dram_tensor("seg", list(pc0_shapes["seg"]), dt.bfloat16, kind="ExternalInput")
    J_d = nc.dram_tensor("J64", [128, WINW], dt.bfloat16, kind="ExternalInput")
    JT_d = nc.dram_tensor("J64T", [64, 1], dt.bfloat16, kind="ExternalInput")
    W1e_d = nc.dram_tensor("W1e", [128, PAY12], dt.float32, kind="ExternalInput")
    W2e_d = nc.dram_tensor("W2e", [2, 128, PAY12], dt.float32, kind="ExternalInput")
    W3e_d = nc.dram_tensor("W3e", [2, 128, PAY3], dt.float32, kind="ExternalInput")
    fcW_d = nc.dram_tensor("fcW", [64, N_CLASSES], dt.float32, kind="ExternalInput")
    b1_d = nc.dram_tensor("b1bc", [128, 256], dt.float32, kind="ExternalInput")
    b2_d = nc.dram_tensor("b2bc", [128, 256], dt.float32, kind="ExternalInput")
    b3_d = nc.dram_tensor("b3bc", [128, 64], dt.float32, kind="ExternalInput")
    fcb_d = nc.dram_tensor("fcbbc", [128, N_CLASSES], dt.float32, kind="ExternalInput")
    out_d = nc.dram_tensor("OUT", [OWN, N_CLASSES], dt.float32, kind="ExternalOutput")

    # ---------------- internals ----------------
    tables = [
        (nc.dram_tensor("table1a", [RA, ROW12], dt.bfloat16, kind="Internal", addr_space="Shared"),
         nc.dram_tensor("table1b", [RB, ROW12], dt.bfloat16, kind="Internal", addr_space="Shared")),
        (nc.dram_tensor("table2a", [RA, ROW12], dt.bfloat16, kind="Internal", addr_space="Shared"),
         nc.dram_tensor("table2b", [RB, ROW12], dt.bfloat16, kind="Internal", addr_space="Shared")),
        (nc.dram_tensor("table3a", [RA, ROW3], dt.bfloat16, kind="Internal", addr_space="Shared"),
         nc.dram_tensor("table3b", [RB, ROW3], dt.bfloat16, kind="Internal", addr_space="Shared")),
    ]
    ag_ins = [
        nc.dram_tensor("agin1", [OWN, ROW12], dt.bfloat16, kind="Internal"),
        nc.dram_tensor("agin2", [OWN, ROW12], dt.bfloat16, kind="Internal"),
        nc.dram_tensor("agin3", [OWN, ROW3], dt.bfloat16, kind="Internal"),
    ]
    edsts = [
        nc.dram_tensor("edst1", [OWN, EROW], dt.bfloat16, kind="Internal"),
        nc.dram_tensor("edst2", [OWN, EROW], dt.bfloat16, kind="Internal"),
        nc.dram_tensor("edst3", [OWN, EROW], dt.bfloat16, kind="Internal"),
    ]
    xs = [
        None,
        nc.dram_tensor("x2", [OWN, 256], dt.float32, kind="Internal"),
        nc.dram_tensor("x3", [OWN, 256], dt.float32, kind="Internal"),
        nc.dram_tensor("x4", [OWN, 128], dt.float32, kind="Internal"),
    ]

    LAYERS = [
        dict(h=H1, F=256, row=ROW12, pay=PAY12, table=tables[0], agin=ag_ins[0],
             edst=edsts[0], b=b1_d, xout=xs[1], We=W1e_d, nkb=1),
        dict(h=H2, F=256, row=ROW12, pay=PAY12, table=tables[1], agin=ag_ins[1],
             edst=edsts[1], b=b2_d, xout=xs[2], We=W2e_d, nkb=2),
        dict(h=H3, F=64, row=ROW3, pay=PAY3, table=tables[2], agin=ag_ins[2],
             edst=edsts[2], b=b3_d, xout=xs[3], We=W3e_d, nkb=2),
    ]

    qctr = [0]

    def next_q():
        q = qctr[0] % NQ
        qctr[0] += 1
        return q

    with tile.TileContext(nc) as tc:
        with tc.tile_pool(name="const", bufs=1) as cpool, \
             tc.tile_pool(name="np_sb", bufs=3) as npool, \
             tc.tile_pool(name="eg", bufs=2) as gpool, \
             tc.tile_pool(name="ep", bufs=3) as epool, \
             tc.tile_pool(name="psum", bufs=1, space="PSUM") as pspool, \
             tc.tile_pool(name="psum_e", bufs=4, space="PSUM") as pspool_e, \
             tc.tile_pool(name="psum_z", bufs=2, space="PSUM") as pspool_z:

            J_t = cpool.tile([128, WINW], dt.bfloat16)
            nc.sync.dma_start(out=J_t[:], in_=J_d.ap())
            JT_t = cpool.tile([64, 1], dt.bfloat16)
            nc.sync.dma_start(out=JT_t[:], in_=JT_d.ap())
            W1e_t = cpool.tile([128, PAY12], dt.float32)
            nc.sync.dma_start(out=W1e_t[:].bitcast(dt.float32r),
                              in_=W1e_d.ap().bitcast(dt.float32r))
            W2e_t = cpool.tile([128, 2 * PAY12], dt.float32)
            for kb in range(2):
                nc.sync.dma_start(out=W2e_t[:, kb * PAY12:(kb + 1) * PAY12].bitcast(dt.float32r),
                                  in_=W2e_d.ap()[kb].bitcast(dt.float32r))
            W3e_t = cpool.tile([128, 2 * PAY3], dt.float32)
            for kb in range(2):
                nc.sync.dma_start(out=W3e_t[:, kb * PAY3:(kb + 1) * PAY3].bitcast(dt.float32r),
                                  in_=W3e_d.ap()[kb].bitcast(dt.float32r))
            fcW_t = cpool.tile([64, N_CLASSES], dt.float32)
            nc.sync.dma_start(out=fcW_t[:], in_=fcW_d.ap())
            from concourse.masks import make_identity
            ident_t = cpool.tile([128, 128], dt.float32)
            make_identity(nc, ident_t[:])
            b_ts = {}
            for nm, d_, wdt in (("b1", b1_d, 256), ("b2", b2_d, 256),
                                ("b3", b3_d, 64), ("fcb", fcb_d, N_CLASSES)):
                t = cpool.tile([128, wdt], dt.float32, tag=f"bias_{nm}")
                nc.sync.dma_start(out=t[:], in_=d_.ap())
                b_ts[nm] = t

            def node_phase(L, li, prev_xw):
                """x @ [W|Wa_src|Wa_dst] for own nodes -> agin rows + edst rows."""
                F, pay, row = L["F"], L["pay"], L["row"]
                nh = L["h"]
                wdmas = []
                f32r = dt.float32r
                for c in range(NCHUNK):
                    ps = pspool.tile([128, pay], dt.float32, tag="np_ps")
                    if li == 0:
                        lhs = npool.tile([128, 128], dt.float32, tag="np_lhs")
                        nc.sync.dma_start(
                            out=lhs[:].bitcast(f32r),
                            in_=x1T_d.ap()[:, c * 128:(c + 1) * 128].bitcast(f32r))
                        nc.tensor.matmul(out=ps[:], lhsT=lhs[:].bitcast(f32r),
                                         rhs=W1e_t[:].bitcast(f32r),
                                         start=True, stop=True)
                    else:
                        xin = xs[li]
                        Wt = W2e_t if li == 1 else W3e_t
                        xc = npool.tile([128, 256], dt.float32, tag="np_xc")
                        rd = nc.sync.dma_start(
                            out=xc[:], in_=xin.ap()[c * 128:(c + 1) * 128, :])
                        if prev_xw is not None:
                            _add_dep_helper(rd.ins, prev_xw[c].ins, sync=True)
                        for kb in range(2):
                            pst = pspool.tile([128, 128], dt.float32, tag="np_tr")
                            nc.tensor.transpose(out=pst[:],
                                                in_=xc[:, kb * 128:(kb + 1) * 128],
                                                identity=ident_t[:])
                            lhs = npool.tile([128, 128], dt.float32, tag="np_lhs")
                            nc.vector.tensor_copy(out=lhs[:].bitcast(f32r), in_=pst[:])
                            nc.tensor.matmul(out=ps[:], lhsT=lhs[:].bitcast(f32r),
                                             rhs=Wt[:, kb * pay:(kb + 1) * pay].bitcast(f32r),
                                             start=(kb == 0), stop=(kb == 1))
                    # epilogue: pack row_sb = [h bf16 | e_src f32] ; edst rows
                    row_sb = npool.tile([128, row], dt.bfloat16, tag="np_row")
                    nc.vector.tensor_copy(out=row_sb[:, 0:F], in_=ps[:, 0:F])
                    rf32 = row_sb[:].bitcast(dt.float32)
                    nc.vector.tensor_copy(out=rf32[:, F // 2:F // 2 + nh],
                                          in_=ps[:, F:F + nh])
                    ed_sb = npool.tile([128, EROW], dt.bfloat16, tag="np_ed")
                    nc.vector.tensor_copy(out=ed_sb[:, 0:nh],
                                          in_=ps[:, F + nh:F + 2 * nh])
                    wdmas.append(nc.sync.dma_start(
                        out=L["agin"].ap()[c * 128:(c + 1) * 128, :],
                        in_=row_sb[:]))
                    wdmas.append(nc.sync.dma_start(
                        out=L["edst"].ap()[c * 128:(c + 1) * 128, :],
                        in_=ed_sb[:]))
                return wdmas

            def edge_phase(L, li, cca, ccb, edst_wd):
                """batched gathers + attention + segment-reduce; writes L["xout"].
                Returns per-chunk xout write DMAs (indexed by chunk)."""
                F, pay, row, nh = L["F"], L["pay"], L["row"], L["h"]
                (tbla, tblb), eds = L["table"], L["edst"]
                xw_by_chunk = [None] * NCHUNK
                off_seg = 0     # tile offset into seg array
                off_g = 0       # int16 col offset into gidx array
                off_e = 0       # int16 col offset into eidx array
                for b in blocks:
                    T, nb, nlo, nhi = b["Tall"], b["nb"], b["nlo"], b["nhi"]
                    c0 = b["c0"]
                    G_t = gpool.tile([128, T, pay], dt.bfloat16, tag="G")
                    S_t = gpool.tile([128, T, WINW], dt.bfloat16, tag="S")
                    sg_t = gpool.tile([128, T], dt.bfloat16, tag="sg")
                    nc.sync.dma_start(out=sg_t[:], in_=seg_d.ap()[:, off_seg:off_seg + T])
                    sgT_t = gpool.tile([64, T * 128], dt.bfloat16, tag="sgT")
                    nc.sync.dma_start(out=sgT_t[:],
                                      in_=segT_d.ap()[:, off_seg * 128:(off_seg + T) * 128])
                    gi_t = gpool.tile([128, T * 8], dt.int16, tag="gi")
                    nc.sync.dma_start(out=gi_t[:], in_=gidx_d.ap()[:, off_g:off_g + T * 8])
                    # e_dst window values for this block's chunks: [64, nb, 8]
                    eap = eds.ap()
                    edW = []
                    for w_ in range(NWIN):
                        t_ = gpool.tile([64, nb, 8], dt.bfloat16, tag=f"edW{w_}")
                        d_ = nc.sync.dma_start(
                            out=t_[:],
                            in_=bass.AP(eap.tensor,
                                        eap.offset + (c0 * 128 + w_ * WINW) * EROW,
                                        [[EROW, 64], [128 * EROW, nb], [1, 8]]))
                        for k in range(nb):
                            _add_dep_helper(d_.ins, edst_wd[c0 + k].ins, sync=True)
                        edW.append(t_)
                    off_seg += T
                    off_g += T * 8

                    glo = nc.gpsimd.dma_gather(
                        out_ap=G_t[:, 0:nlo, :], in_ap=tbla.ap()[:, 0:pay],
                        idxs_ap=gi_t[:, 0:nlo * 8], num_idxs=nlo * 128,
                        num_idxs_reg=nlo * 128, elem_size=pay, elem_step=row,
                        single_packet=False, queue_num=next_q())
                    ghi = nc.gpsimd.dma_gather(
                        out_ap=G_t[:, nlo:T, :], in_ap=tblb.ap()[:, 0:pay],
                        idxs_ap=gi_t[:, nlo * 8:T * 8], num_idxs=nhi * 128,
                        num_idxs_reg=nhi * 128, elem_size=pay, elem_step=row,
                        single_packet=False, queue_num=next_q())
                    _add_dep_helper(glo.ins, cca.ins, sync=True)
                    _add_dep_helper(ghi.ins, ccb.ins, sync=True)

                    # S build: S[p,t,j] = (seg[p,t] == j)
                    in0 = sg_t[:].to_broadcast([128, T, WINW])
                    jap = J_t[:]
                    in1 = bass.AP(jap.tensor, jap.offset,
                                  [jap.ap[0], [0, T], [1, WINW]])
                    nc.vector.tensor_tensor(out=S_t[:], in0=in0, in1=in1,
                                            op=OP.is_equal)
                    # S^T build: ST[j, t, e] = (segT[j,t*128+e] == j)
                    ST_t = gpool.tile([64, T, 128], dt.bfloat16, tag="ST")
                    jt = JT_t[:]
                    in0T = bass.AP(sgT_t[:].tensor, sgT_t[:].offset,
                                   [sgT_t[:].ap[0], [128, T], [1, 128]])
                    in1T = bass.AP(jt.tensor, jt.offset,
                                   [jt.ap[0], [0, T], [0, 128]])
                    nc.vector.tensor_tensor(out=ST_t[:], in0=in0T, in1=in1T,
                                            op=OP.is_equal)

                    if ATH_EDGE < 3:
                        continue
                    # e_dst per edge via matmul: E[e, h] = sum_j ST[j,e] * edW[j,h]
                    E_ps = pspool_z.tile([128, T, nh], dt.float32, tag="ep_E")
                    for t_id, (cl, w_, h_) in enumerate(b["tl"]):
                        nc.tensor.matmul(
                            out=E_ps[:, t_id, :],
                            lhsT=ST_t[:, t_id, :],
                            rhs=edW[w_][:, cl, 0:nh],
                            start=True, stop=True)

                    # edge weights x = exp(leaky(e_src + e_dst))
                    gf32 = G_t[:].bitcast(dt.float32)   # [128, T, pay//2]
                    z_t = gpool.tile([128, T, nh], dt.float32, tag="z")
                    nc.vector.tensor_tensor(
                        out=z_t[:], in0=gf32[:, :, F // 2:F // 2 + nh],
                        in1=E_ps[:], op=OP.add)
                    nc.vector.scalar_tensor_tensor(
                        out=z_t[:], in0=z_t[:], scalar=NEG_SLOPE, in1=z_t[:],
                        op0=OP.mult, op1=OP.max)
                    x_t = gpool.tile([128, T, nh], dt.bfloat16, tag="x")
                    nc.scalar.activation(out=x_t[:], in_=z_t[:], func=AF.Exp)

                    # fold x into G (in place) and write x into cols F..F+nh
                    g4 = bass.AP(G_t[:].tensor, G_t[:].offset,
                                 [G_t[:].ap[0], [pay, T], [CH, nh], [1, CH]])
                    x4 = bass.AP(x_t[:].tensor, x_t[:].offset,
                                 [x_t[:].ap[0], [nh, T], [1, nh], [0, CH]])
                    nc.vector.tensor_tensor(out=g4, in0=g4, in1=x4, op=OP.mult)
                    nc.vector.tensor_copy(out=G_t[:, :, F:F + nh], in_=x_t[:])

                    # matmuls: per chunk psum [128, F+nh]
                    pss = []
                    for cl in range(nb):
                        ep_ps = pspool_e.tile([128, F + nh], dt.float32, tag="ep_ps")
                        pss.append(ep_ps)
                    seq = [(cl, w_) for (cl, w_, h_) in b["tl"]]
                    last_of = {}
                    for i, kw in enumerate(seq):
                        last_of[kw] = i
                    started = {}
                    for t_id, keyw in enumerate(seq):
                        cl, w_ = keyw
                        first = keyw not in started
                        started[keyw] = True
                        nc.tensor.matmul(
                            out=pss[cl][w_ * WINW:(w_ + 1) * WINW, :],
                            lhsT=S_t[:, t_id, :],
                            rhs=G_t[:, t_id, 0:F + nh],
                            start=first, stop=(last_of[keyw] == t_id),
                            tile_position=(0, w_ * WINW),
                            skip_group_check=True)
                    # epilogue per chunk
                    for cl in range(nb if ATH_EDGE >= 4 else 0):
                        c = b["c0"] + cl
                        ps = pss[cl]
                        inv = epool.tile([128, nh], dt.float32, tag="inv")
                        nc.vector.tensor_scalar_add(out=inv[:], in0=ps[:, F:F + nh],
                                                    scalar1=1e-20)
                        nc.vector.reciprocal(out=inv[:], in_=inv[:])
                        if li < 2:
                            o_sb = epool.tile([128, 256], dt.float32, tag="o_sb")
                        else:
                            o_sb = epool.tile([128, 128], dt.float32, tag="o_sb3")
                            nc.vector.memset(o_sb[:, 64:128], 0.0)
                        if int(os.environ.get("ATH_EPB", "1")):
                            iv = inv[:]
                            inv_b = bass.AP(iv.tensor, iv.offset,
                                            [iv.ap[0], [1, nh], [0, CH]])
                            o3 = bass.AP(o_sb[:].tensor, o_sb[:].offset,
                                         [o_sb[:].ap[0], [CH, nh], [1, CH]])
                            ps3 = bass.AP(ps[:].tensor, ps[:].offset,
                                          [ps[:].ap[0], [CH, nh], [1, CH]])
                            nc.vector.tensor_tensor(out=o3, in0=ps3, in1=inv_b,
                                                    op=OP.mult)
                            nc.vector.tensor_tensor(
                                out=o_sb[:, 0:F], in0=o_sb[:, 0:F],
                                in1=b_ts["b" + str(li + 1)][:, 0:F], op=OP.add)
                        else:
                            for h_ in range(nh):
                                nc.vector.scalar_tensor_tensor(
                                    out=o_sb[:, h_ * CH:(h_ + 1) * CH],
                                    in0=ps[:, h_ * CH:(h_ + 1) * CH],
                                    scalar=inv[:, h_:h_ + 1],
                                    in1=b_ts["b" + str(li + 1)][:, h_ * CH:(h_ + 1) * CH],
                                    op0=OP.mult, op1=OP.add)
                        nc.vector.tensor_scalar_max(
                            out=o_sb[:, 0:F], in0=o_sb[:, 0:F], scalar1=0.0)
                        xw_by_chunk[c] = nc.sync.dma_start(
                            out=L["xout"].ap()[c * 128:(c + 1) * 128, :],
                            in_=o_sb[:])
                return xw_by_chunk

            prev_xw = None
            NCA = LSPL // 128   # 25 chunks in half A
            for li, L in enumerate(LAYERS):
                if ATH_STAGE < 2 * li:
                    break
                wdmas = node_phase(L, li, prev_xw)
                agin_wd = wdmas[0::2]
                cca = nc.gpsimd.collective_compute(
                    "AllGather", OP.bypass, replica_groups=rg,
                    ins=[L["agin"].ap()[0:LSPL, :]], outs=[L["table"][0].ap()])
                for wdm in agin_wd[0:NCA]:
                    _add_dep_helper(cca.ins, wdm.ins, sync=True)
                ccb = nc.gpsimd.collective_compute(
                    "AllGather", OP.bypass, replica_groups=rg,
                    ins=[L["agin"].ap()[LSPL:OWN, :]], outs=[L["table"][1].ap()])
                for wdm in agin_wd[NCA:]:
                    _add_dep_helper(ccb.ins, wdm.ins, sync=True)
                if ATH_STAGE < 2 * li + 1:
                    break
                edst_wd = wdmas[1::2]   # edst chunk writes
                prev_xw = edge_phase(L, li, cca, ccb, edst_wd)

            # FC: out = relu3 @ fcW + fcb
            for c in range(NCHUNK if ATH_STAGE >= 6 else 0):
                xc4 = npool.tile([128, 128], dt.float32, tag="fc_xc")
                rd = nc.sync.dma_start(
                    out=xc4[:], in_=xs[3].ap()[c * 128:(c + 1) * 128, 0:128])
                if prev_xw is not None:
                    _add_dep_helper(rd.ins, prev_xw[c].ins, sync=True)
                pst4 = pspool.tile([128, 128], dt.float32, tag="np_tr")
                nc.tensor.transpose(out=pst4[:], in_=xc4[:], identity=ident_t[:])
                lhs = npool.tile([128, 128], dt.float32, tag="fc_lhs")
                nc.vector.tensor_copy(out=lhs[:], in_=pst4[:])
                ps = pspool.tile([128, N_CLASSES], dt.float32, tag="np_ps")
                nc.tensor.matmul(out=ps[:], lhsT=lhs[0:64, :], rhs=fcW_t[:],
                                 start=True, stop=True)
                o_sb = npool.tile([128, N_CLASSES], dt.float32, tag="fc_o")
                nc.vector.tensor_tensor(out=o_sb[:], in0=ps[:],
                                        in1=b_ts["fcb"][:], op=OP.add)
                nc.sync.dma_start(out=out_d.ap()[c * 128:(c + 1) * 128, :],
                                  in_=o_sb[:])

    nc.compile()
    return nc


def prepare(x, edge_index, W1, a1_src, a1_dst, b1, W2, a2_src, a2_dst, b2,
            W3, a3_src, a3_dst, b3, fc_W, fc_b):
    x = np.asarray(x, np.float32)
    edge_index = np.asarray(edge_index)
    blocks, per_core = _host_prep(x, edge_index)

    def ext(Wm, a_s, a_d, nh):
        Wm = np.asarray(Wm, np.float32)
        F = Wm.shape[1]
        A_s = np.zeros((F, nh), np.float32)
        A_d = np.zeros((F, nh), np.float32)
        for h_ in range(nh):
            A_s[h_ * CH:(h_ + 1) * CH, h_] = np.asarray(a_s, np.float32)[h_]
            A_d[h_ * CH:(h_ + 1) * CH, h_] = np.asarray(a_d, np.float32)[h_]
        return np.concatenate([Wm, Wm @ A_s, Wm @ A_d], axis=1)

    W1e = ext(W1, a1_src, a1_dst, H1)                          # [128, 264]
    W2e = ext(W2, a2_src, a2_dst, H2).reshape(2, 128, PAY12)
    W3e = ext(W3, a3_src, a3_dst, H3).reshape(2, 128, PAY3)
    fcWb = np.asarray(fc_W, np.float32)
    J64 = np.broadcast_to(np.arange(WINW, dtype=np.float32), (128, WINW)).astype(bf16)
    J64T = np.arange(WINW, dtype=np.float32)[:, None].astype(bf16)

    def bc(v, w_):
        return np.broadcast_to(np.asarray(v, np.float32)[None, :], (128, w_)).copy()

    xpad = np.zeros((NPAD, IN_DIM), np.float32)
    xpad[:N_NODES] = x

    heads_cfg = dict(pc_shapes={k: per_core[0][k].shape for k in
                                ("gidx", "seg", "segT")})
    nc = _build_program(blocks, heads_cfg)

    in_maps = []
    for r in range(W):
        pc = per_core[r]
        in_maps.append({
            "x1T": np.ascontiguousarray(xpad[r * OWN:(r + 1) * OWN].T),
            "gidx": pc["gidx"], "seg": pc["seg"], "segT": pc["segT"],
            "J64": J64, "J64T": J64T,
            "W1e": W1e, "W2e": W2e, "W3e": W3e, "fcW": fcWb,
            "b1bc": bc(b1, 256), "b2bc": bc(b2, 256), "b3bc": bc(b3, 64),
            "fcbbc": bc(fc_b, N_CLASSES),
        })
    return nc, in_maps


def kernel(x, edge_index, W1, a1_src, a1_dst, b1, W2, a2_src, a2_dst, b2,
           W3, a3_src, a3_dst, b3, fc_W, fc_b):
    nc, in_maps = prepare(x, edge_index, W1, a1_src, a1_dst, b1,
                          W2, a2_src, a2_dst, b2, W3, a3_src, a3_dst, b3,
                          fc_W, fc_b)
    results = _run_spmd(nc, in_maps)
    out = np.concatenate([results[r]["OUT"] for r in range(W)], axis=0)
    return out[:N_NODES].astype(np.float32)


def _run_spmd(nc, in_maps):
    """Execute via PJRT (axon). With ATH_BENCH=1, device_put inputs once and
    time repeated warm executions, recording the best in LAST_EXEC_NS."""
    if not int(os.environ.get("ATH_BENCH", "0")):
        res = bass_utils.run_bass_kernel_spmd(nc, in_maps, core_ids=list(range(W)))
        return res.results

    import time
    import jax
    from jax.sharding import Mesh, PartitionSpec, NamedSharding
    from jax.experimental.shard_map import shard_map
    from concourse import bass2jax, mybir as _mb
    bass2jax.install_neuronx_cc_hook()

    partition_name = nc.partition_id_tensor.name if nc.partition_id_tensor else None
    in_names, out_names, out_avals = [], [], []
    for alloc in nc.m.functions[0].allocations:
        if not isinstance(alloc, _mb.MemoryLocationSet):
            continue
        name = alloc.memorylocations[0].name
        if alloc.kind == "ExternalInput":
            if name != partition_name:
                in_names.append(name)
        elif alloc.kind == "ExternalOutput":
            out_names.append(name)
            out_avals.append(jax.core.ShapedArray(
                tuple(alloc.tensor_shape), _mb.dt.np(alloc.dtype)))
    n_params = len(in_names)
    n_outs = len(out_avals)
    all_in_names = list(in_names) + out_names
    if partition_name is not None:
        all_in_names.append(partition_name)

    def _body(*args):
        operands = list(args)
        if partition_name is not None:
            operands.append(bass2jax.partition_id_tensor())
        return tuple(bass2jax._bass_exec_p.bind(
            *operands, out_avals=tuple(out_avals), in_names=tuple(all_in_names),
            out_names=tuple(out_names), lowering_input_output_aliases=(),
            sim_require_finite=True, sim_require_nnan=True, nc=nc))

    n_cores = len(in_maps)
    devices = jax.devices()[:n_cores]
    mesh = Mesh(np.asarray(devices), ("core",))
    spec = PartitionSpec("core")
    sharded = jax.jit(
        shard_map(_body, mesh=mesh, in_specs=(spec,) * (n_params + n_outs),
                  out_specs=(spec,) * n_outs, check_rep=False),
        keep_unused=True)
    shard = NamedSharding(mesh, spec)
    concat_in = [
        jax.device_put(np.concatenate(
            [np.asarray(in_maps[c][nm]) for c in range(n_cores)], axis=0), shard)
        for nm in in_names]
    zero_outs = [
        jax.device_put(np.zeros((n_cores * a.shape[0], *a.shape[1:]), a.dtype), shard)
        for a in out_avals]

    def _time(reps):
        best, arrs = None, None
        for _ in range(reps):
            t0 = time.perf_counter()
            arrs = sharded(*concat_in, *zero_outs)
            arrs = [o.block_until_ready() for o in arrs]
            dt_ = time.perf_counter() - t0
            best = dt_ if best is None or dt_ < best else best
        return best, arrs

    _time(1)  # warm compile
    t1, out_arrs = _time(4)
    global LAST_EXEC_NS
    LAST_EXEC_NS = int(t1 * 1e9)
    print(f"t1={t1*1e3:.2f} ms", flush=True)
    return [
        {nm: np.asarray(out_arrs[i]).reshape(n_cores, *out_avals[i].shape)[c]
         for i, nm in enumerate(out_names)}
        for c in range(n_cores)
    ]
